# revision 1
# baseline (speedup 1.0000x reference)
"""Trainium2 Bass kernel v2 for Burgers PINN residual (nn_Net_F_78881369358760).

f = u_t + u*u_x - nu*u_xx for a tanh MLP [2,64,64,64,64,1] at 500K points.

Layout: pure data parallel over 8 cores; per core 62 pairs of 1024 points
(pair = [128part, 512free]: tile A units on partitions 0:64, tile B on
64:128). Mid layers use K=128 block-diagonal lhsT so ONE matmul serves both
tiles (4 matmuls/pair/layer). Derivative streams:
  A   = a_l (tanh output)
  XT  = [ax | at~] = sqrt2 * s ⊙ [zx | zt~]   (t~ seed = tcol + b4*xcol
        folds the b4*u_x final term exactly; sqrt2 folds into /sqrt2
        weight variants and makes P = (XTraw_x)^2 = 2 zx^2 for free)
  AXX = s ⊙ (zxx - 2 a zx^2)
zx|zt share one 2-bank PSUM tile, evacuated by ONE 1024-wide ACT copy with
scale sqrt2. The second-derivative stream is SPLIT across two accumulated
matmuls (splitmm): zxx_{l+1} = W(S*zxx_l) - W(R2_l) with
W3 = S*zxx via fused scalar_tensor_tensor reading PSUM, H = A*ZXTx,
R2 = XTx*H (= 2 a s zx^2); the subtraction rides the PE. sigma products
for both first-derivative streams run in one 1024-wide tt with a stride-0
repeat AP on S. a^2 runs on the Pool engine (GPSIMD). L5 (M=1) matmuls
are K=128 M=2 block-diagonal, packed 4 pairs deep into 3 PSUM banks at
tile_position cols {0,32,64,96}; one ACT evac + two DVE ops produce f for
4096 points. Engine budget per core (cost model): DVE 444us (bottleneck,
~88%), ACT 356us, PE 278us, Pool 276us -> sim 505us (vs 677us baseline).
"""
import numpy as np

NU = 0.01 / np.pi
NT = 512            # points per tile
NCORES = 8
NPT_CORE = 62500    # 500000 / 8
NGROUP = 31         # xt DMA groups of 2048 points
NPAIR = 62          # pairs of 1024 points per core


def _split16(a):
    hi = a.astype(np.float16)
    lo = (a.astype(np.float32) - hi.astype(np.float32)).astype(np.float16)
    return hi, lo


DEFAULT_CFG = dict(
    # per-layer engine choices: index 0 = L1, 1..3 = L2..L4 (a2)
    a2_engine=("pool", "pool", "pool", "pool"),  # pool | dve | act
    g1_engine="dve",                             # dve | pool
    # xx-chain mode per mid layer (L2..L4): "splitmm" computes
    #   W3 = S*zxx (stt from psum), H = A*ZXTx, R2 = XTx*H and defers the
    #   subtraction to two accumulated matmuls in the next layer;
    #   "fused" is the old P/M/INNER/AXX chain.
    xx_mode=("splitmm", "splitmm", "splitmm"),
    h_engine=("dve", "dve", "pool"),             # dve | pool
    r2_engine=("dve", "dve", "dve"),             # dve | pool
    # w3 source per mid layer: "dve" = stt from PSUM (1x mode, 658ns);
    # "fuse" = widen the ACT evac to [zx|zt|zxx]*sqrt2 (1536-wide) and
    # compute W3 as a cheap f16 tt (327ns); sqrt2 compensated by the /sqrt2
    # weight variants (wt 5/7, w4 piece 5) on the next layer's W3 matmul.
    w3_engine=("dve", "dve", "dve"),
    l5_mode="3bank4",    # 3bank4 (pxt bufs=1) | 2bank2_act | 2bank2_psum (pxt bufs=2)
    interleave=False,    # interleave the two pairs of a group layer-by-layer
    l1_splitmm=False,    # fold S1 into L2 matmuls (const-mm + A2-mm)
    r2_early=True,       # emit H/R2 before W3 so next-layer zxx mm fires sooner
    # engine for the t-half sigma product, per mid layer; "dvew" = fused with
    # the x-half in one wide DVE tt (original); "dve"/"pool" = separate tiles
    xtt_engine=("dvew", "dvew", "dvew"),
    a2_split=True,       # a^2 as two 256-wide Pool ops + S as two ts ops
    defer_tf=False,      # emit L5 T1/F after the next block's first L1
    xt_split=False,      # XT as two point-halved ops (earlier start vs split S)
    a2_ways=4,           # a2/S split granularity (with a2_split)
    a2_split_layers=(0, 1, 0),  # split only where S-latency is critical (L3)
)


def _build_program(npair=NPAIR, cfg=None):
    cfg = {**DEFAULT_CFG, **(cfg or {})}
    import concourse.bacc as bacc
    import concourse.tile as tile
    from concourse import mybir
    from contextlib import ExitStack

    F16 = mybir.dt.float16
    F32 = mybir.dt.float32
    TANH = mybir.ActivationFunctionType.Tanh
    SQUARE = mybir.ActivationFunctionType.Square
    COPY = mybir.ActivationFunctionType.Copy
    MUL = mybir.AluOpType.mult
    ADD = mybir.AluOpType.add
    SUB = mybir.AluOpType.subtract
    SQRT2 = float(np.sqrt(2.0))

    nc = bacc.Bacc("TRN2", target_bir_lowering=False, debug=False)

    NT2 = 2 * NT
    ngroup = (npair + 1) // 2

    # ---- DRAM I/O ----
    # xt[g, half, 8, NT2]: rows [xhi,thi,xlo,tlo]x2; cols pair0|pair1 of group
    d_xt = nc.dram_tensor("xt", [ngroup, 2, 8, NT2], F16, kind="ExternalInput").ap()
    d_wt0 = nc.dram_tensor("wt0", [128, 64], F16, kind="ExternalInput").ap()
    # Block-diagonal mid lhsT variants, [n, 128, 128]:
    #  0: L2 z   = bd(W1T)            1: L2 x = bd((W1 diag xcol)T)
    #  2: L2 t   = bd((W1 diag tccol)T)  3: L2 xx = bd((W1 diag ccol)T)
    #  4: L3 z/xx= bd(W2T)            5: L3 xt = bd((W2/s2)T)
    #  6: L4 z/xx= bd(W3T)            7: L4 xt = bd((W3/s2)T)
    #  8: L3 xx-r2 = bd(-W2T)         9: L4 xx-r2 = bd(-W3T)
    # 10: -bd((W1 diag xcol)T)  11: -bd((W1 diag tccol)T)  12: -bd((W1 diag ccol)T)
    d_wt = nc.dram_tensor("wt", [13, 128, 128], F16, kind="ExternalInput").ap()
    # K=1 const rows for l1_splitmm: [1, 256] = [cx-bd | ct-bd]
    d_cxt = nc.dram_tensor("cxt", [1, 256], F16, kind="ExternalInput").ap()
    # L5 pieces [128, 12]: {0,1}: u bd2(W4T); {2,3}: ux bd2((W4/s2)T);
    # {4,5}: lin-at bd2((W4/s2)T); {6,7}: lin-xx bd2(-nu*W4T);
    # {8,9}: lin-xx-r2 bd2(+nu*W4T); {10,11}: lin-xx-fused bd2(-nu*W4T/s2)
    d_w4 = nc.dram_tensor("w4", [128, 12], F16, kind="ExternalInput").ap()
    d_bias = nc.dram_tensor("bias", [4, 128, 1], F32, kind="ExternalInput").ap()
    # f out: per pair 2 rows of 512: [npair, 2, NT]
    d_f = nc.dram_tensor("f", [npair, 2, NT], F16, kind="ExternalOutput").ap()

    P0, P1 = slice(0, 64), slice(64, 128)

    with ExitStack() as ctx:
        tc = ctx.enter_context(tile.TileContext(nc))
        consts = ctx.enter_context(tc.tile_pool(name="consts", bufs=1))
        sbx = ctx.enter_context(tc.tile_pool(name="sbx", bufs=4))
        sba = ctx.enter_context(tc.tile_pool(name="sba", bufs=3))
        sbb = ctx.enter_context(tc.tile_pool(name="sbb", bufs=3))
        sbf = ctx.enter_context(tc.tile_pool(name="sbf", bufs=2))
        # psum pools: 8 banks total
        inter = cfg["interleave"]
        if inter:
            cfg["l5_mode"] = "2bank2_psum"
        l5_3bank = cfg["l5_mode"] == "3bank4"
        if inter:
            # z and zxx share one 2-buf ring (tag "z"); zxt 2-deep; L5 2 banks
            pz = ctx.enter_context(tc.tile_pool(name="pz", bufs=2, space="PSUM"))
            pxt = ctx.enter_context(tc.tile_pool(name="pxt", bufs=2, space="PSUM"))
            pxx = pz
            zxx_tag = "z"
        else:
            pz = ctx.enter_context(tc.tile_pool(name="pz", bufs=1, space="PSUM"))
            pxt = ctx.enter_context(tc.tile_pool(
                name="pxt", bufs=1 if l5_3bank else 2, space="PSUM"))
            pxx = ctx.enter_context(tc.tile_pool(
                name="pxx", bufs=2 if l5_3bank else 1, space="PSUM"))
            zxx_tag = "zxx"
        pl5 = ctx.enter_context(tc.tile_pool(name="pl5", bufs=1, space="PSUM"))

        # ---- constants ----
        # L1 needs only wt0 + bias: load those first so pair 0 starts early.
        c_wt0 = consts.tile([128, 64], F16, tag="cwt0")
        nc.sync.dma_start(c_wt0[:], d_wt0[:])
        c_bias = consts.tile([128, 4], F32, tag="cbias")
        nc.sync.dma_start(
            c_bias[:].rearrange("p (l o) -> p l o", o=1),
            d_bias[0:4].transpose([1, 0, 2]))
        nwt = 13 if cfg["l1_splitmm"] else 10
        c_wt = consts.tile([128, nwt * 128], F16, tag="cwt")
        # one DMA: src [i, p, c] -> dest [p, i*128 + c]
        nc.sync.dma_start(
            c_wt[:].rearrange("p (i c) -> p i c", i=nwt),
            d_wt[0:nwt].transpose([1, 0, 2]))
        if cfg["l1_splitmm"]:
            c_cxt = consts.tile([1, 256], F16, tag="ccxt")
            nc.sync.dma_start(c_cxt[:], d_cxt[:])
            c_ones = consts.tile([1, NT], F16, tag="cones")
            nc.vector.memset(c_ones[:], 1.0)
        c_w4 = consts.tile([128, 12], F16, tag="cw4")
        nc.sync.dma_start(c_w4[:], d_w4[:])

        def wt_v(i):
            return c_wt[:, i * 128:(i + 1) * 128]

        def w4p(i):
            return c_w4[:, 2 * i:2 * i + 2]

        def tt(eng, out, i0, i1, op):
            if eng == "pool":
                nc.gpsimd.tensor_tensor(out, i0, i1, op)
            else:
                nc.vector.tensor_tensor(out, i0, i1, op)

        def do_pair(xt_t, C, pj, l5):
            """Generator: yields after L1 and after each mid layer so two
            pairs can be interleaved. pj: pair index within L5 block."""
            # ---- L1 ----
            z1 = pz.tile([128, NT], F32, tag="z")
            for sl, tp in ((P0, (0, 0)), (P1, (64, 64))):
                r8 = slice(sl.start, sl.start + 8)
                nc.tensor.matmul(z1[sl, :], c_wt0[r8, 0:64], xt_t[r8, C],
                                 start=True, stop=True, tile_position=tp)
            A = sba.tile([128, NT], F16, tag="A")
            nc.scalar.activation(A[:], z1[:], TANH, bias=c_bias[:, 0:1], scale=1.0)
            A2 = sbb.tile([128, NT], F16, tag="A2")
            if cfg.get("l1_a2_split") and cfg["a2_engine"][0] != "act":
                NW1 = cfg["a2_ways"]
                for wi in range(NW1):
                    sl_ = slice(wi * NT // NW1, (wi + 1) * NT // NW1)
                    tt(cfg["a2_engine"][0], A2[:, sl_], A[:, sl_], A[:, sl_], MUL)
            elif cfg["a2_engine"][0] == "act":
                nc.scalar.activation(A2[:], A[:], SQUARE, bias=0.0, scale=1.0)
            else:
                tt(cfg["a2_engine"][0], A2[:], A[:], A[:], MUL)
            if cfg["l1_splitmm"]:
                A13 = sba.tile([128, NT], F16, tag="A13")
                tt(cfg["g1_engine"], A13[:], A[:], A2[:], MUL)
                S = G1 = None
            elif cfg.get("l1_a2_split"):
                S = sba.tile([128, NT], F16, tag="S")
                NW1 = cfg["a2_ways"]
                for wi in range(NW1):
                    sl_ = slice(wi * NT // NW1, (wi + 1) * NT // NW1)
                    nc.vector.tensor_scalar(S[:, sl_], A2[:, sl_], -1.0, 1.0, MUL, ADD)
                G1 = sba.tile([128, NT], F16, tag="G1")
                tt(cfg["g1_engine"], G1[:], A[:], S[:], MUL)
            else:
                S = sba.tile([128, NT], F16, tag="S")
                nc.vector.tensor_scalar(S[:], A2[:], -1.0, 1.0, MUL, ADD)
                G1 = sba.tile([128, NT], F16, tag="G1")
                tt(cfg["g1_engine"], G1[:], A[:], S[:], MUL)
                A13 = None
            A2_1 = A2
            yield

            # ---- L2..L4 ----
            # xx carries either ("single", rhs, variant) or ("split", W3s, R2s,
            # v_plain, v_neg) describing how to build the NEXT zxx psum.
            xx = ("single", G1, 3)
            XTX = (S, slice(0, NT))
            XTT = (S, slice(0, NT))
            for l in (0, 1, 2):
                if l == 0:
                    vz, vx, vt = 0, 1, 2
                    rz = A
                else:
                    vz, vx, vt = 4 + 2 * (l - 1), 5 + 2 * (l - 1), 5 + 2 * (l - 1)
                    rz = A
                (rx, rxs), (rt, rts) = XTX, XTT
                vp = 4 + 2 * (l - 1)   # plain bd(WT) of this layer (valid l>=1)
                vn = 8 + (l - 1)       # bd(-WT)
                z = pz.tile([128, NT], F32, tag="z")
                zxt = pxt.tile([128, NT2], F32, tag="zxt")
                zxx = pxx.tile([128, NT], F32, tag=zxx_tag)
                nc.tensor.matmul(z[:], wt_v(vz), rz[:, 0:NT], start=True, stop=True, tile_position=(0, 0))
                if l == 0 and cfg["l1_splitmm"]:
                    nc.tensor.matmul(zxt[:, 0:NT], c_cxt[0:1, 0:128], c_ones[0:1, :], start=True, stop=False, tile_position=(0, 0))
                    nc.tensor.matmul(zxt[:, 0:NT], wt_v(10), A2_1[:, 0:NT], start=False, stop=True, tile_position=(0, 0))
                    nc.tensor.matmul(zxt[:, NT:NT2], c_cxt[0:1, 128:256], c_ones[0:1, :], start=True, stop=False, tile_position=(0, 0))
                    nc.tensor.matmul(zxt[:, NT:NT2], wt_v(11), A2_1[:, 0:NT], start=False, stop=True, tile_position=(0, 0))
                    nc.tensor.matmul(zxx[:], wt_v(3), A[:, 0:NT], start=True, stop=False, tile_position=(0, 0))
                    nc.tensor.matmul(zxx[:], wt_v(12), A13[:, 0:NT], start=False, stop=True, tile_position=(0, 0))
                else:
                    nc.tensor.matmul(zxt[:, 0:NT], wt_v(vx), rx[:, rxs], start=True, stop=True, tile_position=(0, 0))
                    nc.tensor.matmul(zxt[:, NT:NT2], wt_v(vt), rt[:, rts], start=True, stop=True, tile_position=(0, 0))
                    if xx[0] == "single":
                        nc.tensor.matmul(zxx[:], wt_v(xx[2]), xx[1][:, 0:NT], start=True, stop=True, tile_position=(0, 0))
                    else:
                        nc.tensor.matmul(zxx[:], wt_v(xx[3]), xx[1][:, 0:NT], start=True, stop=False, tile_position=(0, 0))
                        nc.tensor.matmul(zxx[:], wt_v(xx[4]), xx[2][:, 0:NT], start=False, stop=True, tile_position=(0, 0))

                A = sba.tile([128, NT], F16, tag="A")
                nc.scalar.activation(A[:], z[:], TANH, bias=c_bias[:, l + 1:l + 2], scale=1.0)
                w3_fuse = cfg["w3_engine"][l] == "fuse" and not inter
                ZXTt = sbx.tile([128, NT2], F16, tag="ZXT")
                nc.scalar.activation(ZXTt[:], zxt[:], COPY, bias=0.0, scale=SQRT2)
                ZXT = ZXTt[:]
                if w3_fuse:
                    zxxs = sbb.tile([128, NT], F16, tag="zxxs")
                    nc.scalar.activation(zxxs[:], zxx[:], COPY, bias=0.0, scale=1.0)
                A2 = sbb.tile([128, NT], F16, tag="A2")
                NW = cfg["a2_ways"]
                NH = NT // NW
                a2sp = cfg["a2_split"] and (cfg.get("a2_split_layers") or (1, 1, 1))[l]
                if a2sp:
                    for wi in range(NW):
                        sl_ = slice(wi * NH, (wi + 1) * NH)
                        tt(cfg["a2_engine"][l + 1], A2[:, sl_], A[:, sl_], A[:, sl_], MUL)
                elif cfg["a2_engine"][l + 1] == "act":
                    nc.scalar.activation(A2[:], A[:], SQUARE, bias=0.0, scale=1.0)
                else:
                    tt(cfg["a2_engine"][l + 1], A2[:], A[:], A[:], MUL)
                splitmm = cfg["xx_mode"][l] == "splitmm"
                if splitmm and cfg.get("h_early"):
                    H = sbb.tile([128, NT], F16, tag="H")
                    tt(cfg["h_engine"][l], H[:], A[:], ZXT[:, 0:NT], MUL)
                S = sba.tile([128, NT], F16, tag="S")
                if a2sp:
                    for wi in range(NW):
                        sl_ = slice(wi * NH, (wi + 1) * NH)
                        nc.vector.tensor_scalar(S[:, sl_], A2[:, sl_], -1.0, 1.0, MUL, ADD)
                else:
                    nc.vector.tensor_scalar(S[:], A2[:], -1.0, 1.0, MUL, ADD)
                if cfg["xtt_engine"][l] == "dvew":
                    XT = sbx.tile([128, NT2], F16, tag="XT")
                    if cfg["xt_split"] and (cfg.get("xt_split_layers") or (1, 1, 1))[l]:
                        NQ = NT // 2
                        for wi in range(2):
                            ps_ = slice(wi * NQ, (wi + 1) * NQ)
                            s_rep = S[:, ps_].unsqueeze(1).broadcast_to((128, 2, NQ))
                            nc.vector.tensor_tensor(
                                XT[:].rearrange("p (a b) -> p a b", a=2)[:, :, ps_], s_rep,
                                ZXT[:].rearrange("p (a b) -> p a b", a=2)[:, :, ps_], MUL)
                    else:
                        s_rep = S[:].unsqueeze(1).broadcast_to((128, 2, NT))
                        nc.vector.tensor_tensor(
                            XT[:].rearrange("p (a b) -> p a b", a=2), s_rep,
                            ZXT[:].rearrange("p (a b) -> p a b", a=2), MUL)
                    XTX = (XT, slice(0, NT))
                    XTT = (XT, slice(NT, NT2))
                else:
                    XTxs = sbx.tile([128, NT], F16, tag="XTxs")
                    nc.vector.tensor_tensor(XTxs[:], S[:], ZXT[:, 0:NT], MUL)
                    XTts = sbx.tile([128, NT], F16, tag="XTts")
                    tt(cfg["xtt_engine"][l], XTts[:], S[:], ZXT[:, NT:NT2], MUL)
                    XTX = (XTxs, slice(0, NT))
                    XTT = (XTts, slice(0, NT))
                nvp = (4 + 2 * l) if l < 2 else None
                nvn = 8 + l if l < 2 else None
                lpiece = 3
                if splitmm:
                    r2e = (cfg.get("r2_early_layers") or (cfg.get("r2_early"),) * 3)[l]
                    if r2e:
                        if not cfg.get("h_early"):
                            H = sbb.tile([128, NT], F16, tag="H")
                            tt(cfg["h_engine"][l], H[:], A[:], ZXT[:, 0:NT], MUL)
                        R2 = sbb.tile([128, NT], F16, tag="R2")
                        tt(cfg["r2_engine"][l], R2[:], XTX[0][:, XTX[1]], H[:], MUL)
                        W3s = sba.tile([128, NT], F16, tag="W3s")
                        if w3_fuse:
                            nc.vector.tensor_tensor(W3s[:], S[:], zxxs[:], MUL)
                        else:
                            nc.vector.scalar_tensor_tensor(W3s[:], S[:], 1.0, zxx[:], MUL, MUL)
                        xx = ("split", W3s, R2, nvp, nvn, lpiece)
                    elif cfg.get("w3_last2"):
                        H = sbb.tile([128, NT], F16, tag="H")
                        tt(cfg["h_engine"][l], H[:], A[:], ZXT[:, 0:NT], MUL)
                        W3s = sba.tile([128, NT], F16, tag="W3s")
                        nc.vector.scalar_tensor_tensor(W3s[:], S[:], 1.0, zxx[:], MUL, MUL)
                        R2 = sbb.tile([128, NT], F16, tag="R2")
                        tt(cfg["r2_engine"][l], R2[:], XTX[0][:, XTX[1]], H[:], MUL)
                        xx = ("split", W3s, R2, nvp, nvn, lpiece)
                    else:
                        W3s = sba.tile([128, NT], F16, tag="W3s")
                        if cfg["w3_engine"][l] == "evac":
                            zxxs = sbb.tile([128, NT], F16, tag="zxxs")
                            nc.scalar.activation(zxxs[:], zxx[:], COPY, bias=0.0, scale=1.0)
                            tt("dve", W3s[:], S[:], zxxs[:], MUL)
                        else:
                            nc.vector.scalar_tensor_tensor(W3s[:], S[:], 1.0, zxx[:], MUL, MUL)
                        if not cfg.get("h_early"):
                            H = sbb.tile([128, NT], F16, tag="H")
                            tt(cfg["h_engine"][l], H[:], A[:], ZXT[:, 0:NT], MUL)
                        R2 = sbb.tile([128, NT], F16, tag="R2")
                        tt(cfg["r2_engine"][l], R2[:], XTX[0][:, XTX[1]], H[:], MUL)
                        xx = ("split", W3s, R2, nvp, nvn, lpiece)
                else:
                    P = sbb.tile([128, NT], F16, tag="P")
                    nc.vector.tensor_tensor(P[:], ZXT[:, 0:NT], ZXT[:, 0:NT], MUL)
                    M = sbb.tile([128, NT], F16, tag="M")
                    nc.vector.tensor_tensor(M[:], A[:], P[:], MUL)
                    INNER = sbb.tile([128, NT], F16, tag="INNER")
                    nc.vector.scalar_tensor_tensor(INNER[:], M[:], -1.0, zxx[:], MUL, ADD)
                    AXX = sba.tile([128, NT], F16, tag="AXX")
                    nc.vector.tensor_tensor(AXX[:], S[:], INNER[:], MUL)
                    xx = ("single", AXX, nvp)
                yield

            # ---- L5 ----
            if cfg["l5_mode"] == "3bank4":
                cp = 32 * pj
                O = slice(cp, cp + 2)
                u_t, ux_t, ux_cp = l5["u"], l5["ux"], cp
            else:
                # 2-bank: u rows {32j}, ux rows {64+32j} of the SAME bank
                cp = 32 * pj
                O = slice(cp, cp + 2)
                u_t, ux_t, ux_cp = l5["u"], l5["u"], 64 + 32 * pj
            OX = slice(ux_cp, ux_cp + 2)
            nc.tensor.matmul(u_t[O, :], w4p(0), A[:, 0:NT], start=True, stop=True, tile_position=(0, cp))
            nc.tensor.matmul(ux_t[OX, :], w4p(1), XTX[0][:, XTX[1]], start=True, stop=True, tile_position=(0, ux_cp))
            if xx[0] == "split":
                nc.tensor.matmul(l5["lin"][O, :], w4p(2), XTT[0][:, XTT[1]], start=True, stop=False, tile_position=(0, cp))
                nc.tensor.matmul(l5["lin"][O, :], w4p(xx[5]), xx[1][:, 0:NT], start=False, stop=False, tile_position=(0, cp))
                nc.tensor.matmul(l5["lin"][O, :], w4p(4), xx[2][:, 0:NT], start=False, stop=True, tile_position=(0, cp))
            else:
                nc.tensor.matmul(l5["lin"][O, :], w4p(2), XTT[0][:, XTT[1]], start=True, stop=False, tile_position=(0, cp))
                nc.tensor.matmul(l5["lin"][O, :], w4p(3), xx[1][:, 0:NT], start=False, stop=True, tile_position=(0, cp))

        def flush_us(l5, nblk):
            R = slice(0, 32 * (nblk - 1) + 2)
            US = sbf.tile([98, NT], F16, tag="US")
            nc.scalar.activation(US[R, :], l5["u"][R, :], COPY, bias=0.0, scale=1.0)
            return US

        def flush_tf(l5, nblk, pair0, US):
            R = slice(0, 32 * (nblk - 1) + 2)
            T1 = sbf.tile([98, NT], F16, tag="T1")
            nc.vector.tensor_tensor(T1[R, :], US[R, :], l5["ux"][R, :], MUL)
            F = sbf.tile([128, NT], F16, tag="F")
            nc.vector.tensor_tensor(F[R, :], T1[R, :], l5["lin"][R, :], ADD)
            for j in range(nblk):
                nc.sync.dma_start(d_f[pair0 + j], F[32 * j:32 * j + 2, :])

        def flush_l5(l5, nblk, pair0):
            R = slice(0, 32 * (nblk - 1) + 2)
            if cfg["l5_mode"] == "3bank4":
                US = sbf.tile([98, NT], F16, tag="US")
                nc.scalar.activation(US[R, :], l5["u"][R, :], COPY, bias=0.0, scale=1.0)
                T1 = sbf.tile([98, NT], F16, tag="T1")
                nc.vector.tensor_tensor(T1[R, :], US[R, :], l5["ux"][R, :], MUL)
                F = sbf.tile([128, NT], F16, tag="F")
                nc.vector.tensor_tensor(F[R, :], T1[R, :], l5["lin"][R, :], ADD)
            elif cfg["l5_mode"] == "2bank2_act":
                US = sbf.tile([34, NT], F16, tag="US")
                nc.scalar.activation(US[R, :], l5["u"][R, :], COPY, bias=0.0, scale=1.0)
                UXS = sbf.tile([34, NT], F16, tag="UXS")
                nc.scalar.activation(UXS[R, :], l5["u"][64:64 + 32 * (nblk - 1) + 2, :],
                                     COPY, bias=0.0, scale=1.0)
                T1 = sbf.tile([34, NT], F16, tag="T1")
                nc.vector.tensor_tensor(T1[R, :], US[R, :], UXS[R, :], MUL)
                F = sbf.tile([34, NT], F16, tag="F")
                nc.vector.tensor_tensor(F[R, :], T1[R, :], l5["lin"][R, :], ADD)
            else:  # 2bank2_psum
                US = sbf.tile([34, NT], F16, tag="US")
                nc.scalar.activation(US[R, :], l5["u"][R, :], COPY, bias=0.0, scale=1.0)
                T1 = sbf.tile([34, NT], F16, tag="T1")
                nc.vector.tensor_tensor(T1[R, :], US[R, :],
                                        l5["u"][64:64 + 32 * (nblk - 1) + 2, :], MUL)
                F = sbf.tile([34, NT], F16, tag="F")
                nc.vector.tensor_tensor(F[R, :], T1[R, :], l5["lin"][R, :], ADD)
            # rows {32j, 32j+1} -> d_f[pair0 + j]
            for j in range(nblk):
                nc.sync.dma_start(d_f[pair0 + j], F[32 * j:32 * j + 2, :])

        def drive(gens):
            done = [False] * len(gens)
            while not all(done):
                for i, g in enumerate(gens):
                    if not done[i]:
                        try:
                            next(g)
                        except StopIteration:
                            done[i] = True

        LB = 4 if cfg["l5_mode"] == "3bank4" else 2
        xt_tiles = {}
        pending = None
        pi = 0
        while pi < npair:
            nblk = min(LB, npair - pi)
            l5u = pl5.tile([128, NT], F32, tag="l5u")
            l5lin = pl5.tile([128, NT], F32, tag="l5lin")
            l5 = dict(u=l5u, lin=l5lin)
            if cfg["l5_mode"] == "3bank4":
                l5ux = pl5.tile([128, NT], F32, tag="l5ux")
                l5["ux"] = l5ux
            gens = []
            for j in range(nblk):
                p = pi + j
                g, half = p // 2, p % 2
                if half == 0:
                    xt_t = sbx.tile([128, NT2], F16, tag="xt")
                    nc.sync.dma_start(xt_t[0:8, :], d_xt[g, 0])
                    nc.sync.dma_start(xt_t[64:72, :], d_xt[g, 1])
                    xt_tiles[g] = xt_t
                xt_t = xt_tiles[g]
                # pair p uses cols: pair0 -> 0:NT, pair1 -> NT:NT2 of group tiles
                C = slice(half * NT, half * NT + NT)
                gens.append(do_pair(xt_t, C, j, l5))
            if inter:
                drive(gens)
            elif cfg["defer_tf"]:
                next(gens[0])          # L1 of first pair
                if pending is not None:
                    flush_tf(*pending)
                    pending = None
                for g_ in gens:
                    drive([g_])
            else:
                for g_ in gens:
                    drive([g_])
            if cfg["defer_tf"]:
                US = flush_us(l5, nblk)
                pending = (l5, nblk, pi, US)
            else:
                flush_l5(l5, nblk, pi)
            pi += nblk
        if pending is not None:
            flush_tf(*pending)

    nc.compile()
    return nc


def _host_prep(x, t, W0, b0, W1, b1, W2, b2, W3, b3, W4, b4,
               npair=NPAIR, npt_core=NPT_CORE):
    ngroup = (npair + 1) // 2
    pad_core = ngroup * 4 * NT
    n_total = NCORES * npt_core
    xf = np.asarray(x).reshape(-1).astype(np.float32)[:n_total]
    tf = np.asarray(t).reshape(-1).astype(np.float32)[:n_total]

    W0 = np.asarray(W0, np.float32)
    W1 = np.asarray(W1, np.float32)
    W2 = np.asarray(W2, np.float32)
    W3 = np.asarray(W3, np.float32)
    W4 = np.asarray(W4, np.float32)
    b4v = float(np.asarray(b4).reshape(-1)[0])
    s2 = np.float32(np.sqrt(2.0))

    xcol = W0[:, 0]
    tccol = W0[:, 1] + np.float32(b4v) * W0[:, 0]   # t~ seed folds b4*u_x
    ccol = -2.0 * W0[:, 0] ** 2

    def bd(M):
        out = np.zeros((128, 128), np.float16)
        out[0:64, 0:64] = M.astype(np.float16)
        out[64:128, 64:128] = M.astype(np.float16)
        return out

    wt = np.zeros((13, 128, 128), np.float16)
    wt[0] = bd(W1.T)
    wt[1] = bd(xcol[:, None] * W1.T)
    wt[2] = bd(tccol[:, None] * W1.T)
    wt[3] = bd(ccol[:, None] * W1.T)
    wt[4] = bd(W2.T)
    wt[5] = bd(W2.T / s2)
    wt[6] = bd(W3.T)
    wt[7] = bd(W3.T / s2)
    wt[8] = bd(-W2.T)
    wt[9] = bd(-W3.T)
    wt[10] = bd(-(xcol[:, None] * W1.T))
    wt[11] = bd(-(tccol[:, None] * W1.T))
    wt[12] = bd(-(ccol[:, None] * W1.T))

    # K=1 const rows: cx = W1 @ (diag(xcol) 1) = W1 @ xcol, ct = W1 @ tccol
    cx = (W1 @ xcol).astype(np.float16)
    ct = (W1 @ tccol).astype(np.float16)
    cxt = np.zeros((1, 256), np.float16)
    cxt[0, 0:64] = cx
    cxt[0, 64:128] = cx
    cxt[0, 128:192] = ct
    cxt[0, 192:256] = ct

    # L1 exact product lhsT (same as v1)
    W0Thi, W0Tlo = _split16(W0.T)
    wt0_half = np.concatenate([W0Thi, W0Tlo, W0Tlo, W0Thi], 0)  # [8, 64]
    wt0 = np.zeros((128, 64), np.float16)
    wt0[0:8] = wt0_half
    wt0[64:72] = wt0_half

    # L5 pieces: bd2(v) = [128,2]: col0 = [v;0], col1 = [0;v]
    def bd2(v):
        out = np.zeros((128, 2), np.float16)
        out[0:64, 0] = v.astype(np.float16)
        out[64:128, 1] = v.astype(np.float16)
        return out

    w4r = W4.reshape(-1)
    w4 = np.concatenate([
        bd2(w4r),                      # u
        bd2(w4r / s2),                 # ux
        bd2(w4r / s2),                 # lin: at~ piece
        bd2(np.float32(-NU) * w4r),    # lin: xx piece (W3s plain or AXX)
        bd2(np.float32(NU) * w4r),     # lin: xx R2 piece (splitmm)
        bd2(np.float32(-NU) * w4r / s2),  # lin: xx piece for sqrt2-scaled W3s
    ], axis=1)  # [128, 12]

    def dup_col(v):
        out = np.zeros((128, 1), np.float32)
        out[0:64, 0] = v
        out[64:128, 0] = v
        return out

    bias = np.stack([dup_col(np.asarray(b, np.float32).reshape(-1))
                     for b in (b0, b1, b2, b3)])

    in_maps = []
    for c in range(NCORES):
        xs = np.zeros(pad_core, np.float32)
        ts_ = np.zeros(pad_core, np.float32)
        xs[:npt_core] = xf[c * npt_core:(c + 1) * npt_core]
        ts_[:npt_core] = tf[c * npt_core:(c + 1) * npt_core]
        xhi, xlo = _split16(xs)
        thi, tlo = _split16(ts_)
        rows = np.stack([xhi, thi, xlo, tlo, xhi, thi, xlo, tlo])  # [8, pad]
        r4 = rows.reshape(8, ngroup, 4, NT)  # tiles: A0,B0,A1,B1
        xt = np.zeros((ngroup, 2, 8, 2 * NT), np.float16)
        xt[:, 0, :, 0:NT] = np.transpose(r4[:, :, 0], (1, 0, 2))
        xt[:, 0, :, NT:] = np.transpose(r4[:, :, 2], (1, 0, 2))
        xt[:, 1, :, 0:NT] = np.transpose(r4[:, :, 1], (1, 0, 2))
        xt[:, 1, :, NT:] = np.transpose(r4[:, :, 3], (1, 0, 2))
        in_maps.append(dict(xt=xt, wt=wt, wt0=wt0, w4=w4, bias=bias, cxt=cxt))
    return in_maps


def _gather(results, npair=NPAIR, npt_core=NPT_CORE):
    outs = []
    for c in range(NCORES):
        f = results[c]["f"].astype(np.float32)  # [npair, 2, NT]
        # pair p covers: row0 = tile A (pts 4g*NT + ...), row1 = tile B.
        # point order per group g: A0, B0, A1, B1 each NT:
        #   pair (g,0): rows (A0, B0); pair (g,1): rows (A1, B1)
        f = f.reshape(npair // 2, 2, 2, NT)       # [g, half, AB, NT]
        f = np.transpose(f, (0, 1, 2, 3)).reshape(npair // 2, 4 * NT)
        outs.append(f.reshape(-1)[:npt_core])
    return np.concatenate(outs)[:, None]


_CACHED_NC = None


def kernel(**inputs):
    global _CACHED_NC
    import sys
    if "/opt/trn_rl_repo" not in sys.path:
        sys.path.insert(0, "/opt/trn_rl_repo")
    from concourse.bass_utils import run_bass_kernel_spmd

    if _CACHED_NC is None:
        _CACHED_NC = _build_program()
    nc = _CACHED_NC
    in_maps = _host_prep(**inputs)
    res = run_bass_kernel_spmd(nc, in_maps, list(range(NCORES)))
    return _gather(res.results)


if __name__ == "__main__":
    rng = np.random.default_rng(0)
    LAYERS = [2, 64, 64, 64, 64, 1]
    inp = dict(
        x=rng.standard_normal((500000, 1)).astype(np.float32),
        t=rng.random((500000, 1)).astype(np.float32),
    )
    for i in range(5):
        inp[f"W{i}"] = (rng.standard_normal((LAYERS[i + 1], LAYERS[i]))
                        / np.sqrt(LAYERS[i])).astype(np.float32)
        inp[f"b{i}"] = np.zeros(LAYERS[i + 1], np.float32)
    out = kernel(**inp)
    print("out", out.shape, out.dtype, np.abs(out).max())



# revision 25
# speedup vs baseline: 1.3169x; 1.3169x over previous
"""Trainium2 Bass kernel v2 for Burgers PINN residual (nn_Net_F_78881369358760).

f = u_t + u*u_x - nu*u_xx for a tanh MLP [2,64,64,64,64,1] at 500K points.

Layout: pure data parallel over 8 cores; per core 62 pairs of 1024 points
(pair = [128part, 512free]: tile A units on partitions 0:64, tile B on
64:128). Mid layers use K=128 block-diagonal lhsT so ONE matmul serves both
tiles (4 matmuls/pair/layer). Derivative streams:
  A   = a_l (tanh output)
  XT  = [ax | at~] = sqrt2 * s ⊙ [zx | zt~]   (t~ seed = tcol + b4*xcol
        folds the b4*u_x final term exactly; sqrt2 folds into /sqrt2
        weight variants and makes P = (XTraw_x)^2 = 2 zx^2 for free)
  AXX = s ⊙ (zxx - 2 a zx^2)
zx|zt share one 2-bank PSUM tile, evacuated by ONE 1024-wide ACT copy with
scale sqrt2. The second-derivative stream on L2/L3 is SPLIT across two
accumulated matmuls (splitmm): zxx_{l+1} = W(S*zxx_l) - W(R2_l) with
W3 = S*zxx via fused scalar_tensor_tensor reading PSUM, H = A*ZXTx,
R2 = XTx*H (= 2 a s zx^2); the subtraction rides the PE. L4 uses the
FUSED chain (P/M/INNER/AXX) instead -- one fewer L5-lin matmul and a
shorter L4->L5 dependency tail; HW-measured ~7% faster than splitmm-L4
despite more DVE ops. sigma products for both first-derivative streams
run in one 1024-wide tt with a stride-0 repeat AP on S (runs in DVE 2x
mode on HW). a^2 runs on the Pool engine unsplit. L5 (M=1) matmuls are
K=128 M=2 block-diagonal, packed 4 pairs deep into 3 PSUM banks at
tile_position cols {0,32,64,96}; one ACT evac + two DVE ops produce f
for 4096 points. First xt group DMA is hoisted ahead of the 300KB+
weight-variant DMA (shared in-order queue).

HW profile (NTFF, throttled trn2): total ~620us/core, DVE busy ~585us
(93%, the bottleneck; ~130us of that is in-instruction operand stalls),
PE ~420us, ACT ~375us, Pool ~290us. Offloading DVE ops to Pool/ACT or
reordering (h_early, evac_split, xtt splits, deeper tile-pool bufs) all
HW-measured WORSE: producers on slow in-order queues lengthen consumer
stalls. Cost-model sim ~528us tracks ordering effects but misprices
engine rates (PE ~2.4x, DVE ~1.7x slower on HW than modeled).
"""
import numpy as np

NU = 0.01 / np.pi
NT = 512            # points per tile
NCORES = 8
NPT_CORE = 62500    # 500000 / 8
NGROUP = 31         # xt DMA groups of 2048 points
NPAIR = 62          # pairs of 1024 points per core


def _split16(a):
    hi = a.astype(np.float16)
    lo = (a.astype(np.float32) - hi.astype(np.float32)).astype(np.float16)
    return hi, lo


DEFAULT_CFG = dict(
    # per-layer engine choices: index 0 = L1, 1..3 = L2..L4 (a2)
    a2_engine=("pool", "pool", "pool", "pool"),  # pool | dve | act
    g1_engine="dve",                             # dve | pool
    # xx-chain mode per mid layer (L2..L4): "splitmm" computes
    #   W3 = S*zxx (stt from psum), H = A*ZXTx, R2 = XTx*H and defers the
    #   subtraction to two accumulated matmuls in the next layer;
    #   "fused" is the old P/M/INNER/AXX chain.
    xx_mode=("splitmm", "splitmm", "fused"),
    h_engine=("dve", "dve", "pool"),             # dve | pool
    r2_engine=("dve", "dve", "dve"),             # dve | pool
    # w3 source per mid layer: "dve" = stt from PSUM (1x mode, 658ns);
    # "fuse" = widen the ACT evac to [zx|zt|zxx]*sqrt2 (1536-wide) and
    # compute W3 as a cheap f16 tt (327ns); sqrt2 compensated by the /sqrt2
    # weight variants (wt 5/7, w4 piece 5) on the next layer's W3 matmul.
    w3_engine=("dve", "dve", "dve"),
    l5_mode="3bank4",    # 3bank4 (pxt bufs=1) | 2bank2_act | 2bank2_psum (pxt bufs=2)
    interleave=False,    # interleave the two pairs of a group layer-by-layer
    l1_splitmm=False,    # fold S1 into L2 matmuls (const-mm + A2-mm)
    r2_early=False,      # emit H/R2 before W3 so next-layer zxx mm fires sooner
    w3_last2=True,       # H first, then W3s, then R2
    # engine for the t-half sigma product, per mid layer; "dvew" = fused with
    # the x-half in one wide DVE tt (original); "dve"/"pool" = separate tiles
    xtt_engine=("dvew", "dvew", "dvew"),
    a2_split=True,       # a^2 as two 256-wide Pool ops + S as two ts ops
    defer_tf=False,      # emit L5 T1/F after the next block's first L1
    xt_split=False,      # XT as two point-halved ops (earlier start vs split S)
    a2_ways=4,           # a2/S split granularity (with a2_split)
    a2_split_layers=(0, 0, 0),  # split only where S-latency is critical
    # 3bank4 flush: evac ux/lin PSUM->SBUF via ACT so T1/F run in DVE 2x
    # mode (or on Pool) instead of 1x PSUM-operand mode.
    flush_evac=False,
    flush_engine=("dve", "dve"),   # T1, F engines (dve | pool)
    # engines for the fused-xx chain ops (P, M, INNER, AXX)
    p_engine="dve",       # dve | act | pool
    m_engine="dve",       # dve | pool
    inner_engine="dve",   # dve | pool  (stt reading PSUM)
    axx_engine="dve",     # dve | pool
    # SBUF tile-pool depths (scheduling slack)
    bufs_sbx=4, bufs_sba=3, bufs_sbb=3, bufs_sbf=2,
)


def _build_program(npair=NPAIR, cfg=None):
    cfg = {**DEFAULT_CFG, **(cfg or {})}
    import concourse.bacc as bacc
    import concourse.tile as tile
    from concourse import mybir
    from contextlib import ExitStack

    F16 = mybir.dt.float16
    F32 = mybir.dt.float32
    TANH = mybir.ActivationFunctionType.Tanh
    SQUARE = mybir.ActivationFunctionType.Square
    COPY = mybir.ActivationFunctionType.Copy
    MUL = mybir.AluOpType.mult
    ADD = mybir.AluOpType.add
    SUB = mybir.AluOpType.subtract
    SQRT2 = float(np.sqrt(2.0))

    nc = bacc.Bacc("TRN2", target_bir_lowering=False, debug=False)

    NT2 = 2 * NT
    ngroup = (npair + 1) // 2

    # ---- DRAM I/O ----
    # xt[g, half, 8, NT2]: rows [xhi,thi,xlo,tlo]x2; cols pair0|pair1 of group
    d_xt = nc.dram_tensor("xt", [ngroup, 2, 8, NT2], F16, kind="ExternalInput").ap()
    d_wt0 = nc.dram_tensor("wt0", [128, 64], F16, kind="ExternalInput").ap()
    # Block-diagonal mid lhsT variants, [n, 128, 128]:
    #  0: L2 z   = bd(W1T)            1: L2 x = bd((W1 diag xcol)T)
    #  2: L2 t   = bd((W1 diag tccol)T)  3: L2 xx = bd((W1 diag ccol)T)
    #  4: L3 z/xx= bd(W2T)            5: L3 xt = bd((W2/s2)T)
    #  6: L4 z/xx= bd(W3T)            7: L4 xt = bd((W3/s2)T)
    #  8: L3 xx-r2 = bd(-W2T)         9: L4 xx-r2 = bd(-W3T)
    # 10: -bd((W1 diag xcol)T)  11: -bd((W1 diag tccol)T)  12: -bd((W1 diag ccol)T)
    d_wt = nc.dram_tensor("wt", [13, 128, 128], F16, kind="ExternalInput").ap()
    # K=1 const rows for l1_splitmm: [1, 256] = [cx-bd | ct-bd]
    d_cxt = nc.dram_tensor("cxt", [1, 256], F16, kind="ExternalInput").ap()
    # L5 pieces [128, 12]: {0,1}: u bd2(W4T); {2,3}: ux bd2((W4/s2)T);
    # {4,5}: lin-at bd2((W4/s2)T); {6,7}: lin-xx bd2(-nu*W4T);
    # {8,9}: lin-xx-r2 bd2(+nu*W4T); {10,11}: lin-xx-fused bd2(-nu*W4T/s2)
    d_w4 = nc.dram_tensor("w4", [128, 12], F16, kind="ExternalInput").ap()
    d_bias = nc.dram_tensor("bias", [4, 128, 1], F32, kind="ExternalInput").ap()
    # f out: per pair 2 rows of 512: [npair, 2, NT]
    d_f = nc.dram_tensor("f", [npair, 2, NT], F16, kind="ExternalOutput").ap()

    P0, P1 = slice(0, 64), slice(64, 128)

    with ExitStack() as ctx:
        tc = ctx.enter_context(tile.TileContext(nc))
        consts = ctx.enter_context(tc.tile_pool(name="consts", bufs=1))
        sbx = ctx.enter_context(tc.tile_pool(name="sbx", bufs=cfg["bufs_sbx"]))
        sba = ctx.enter_context(tc.tile_pool(name="sba", bufs=cfg["bufs_sba"]))
        sbb = ctx.enter_context(tc.tile_pool(name="sbb", bufs=cfg["bufs_sbb"]))
        sbf = ctx.enter_context(tc.tile_pool(name="sbf", bufs=cfg["bufs_sbf"]))
        # psum pools: 8 banks total
        inter = cfg["interleave"]
        if inter:
            cfg["l5_mode"] = "2bank2_psum"
        l5_3bank = cfg["l5_mode"] == "3bank4"
        if inter:
            # z and zxx share one 2-buf ring (tag "z"); zxt 2-deep; L5 2 banks
            pz = ctx.enter_context(tc.tile_pool(name="pz", bufs=2, space="PSUM"))
            pxt = ctx.enter_context(tc.tile_pool(name="pxt", bufs=2, space="PSUM"))
            pxx = pz
            zxx_tag = "z"
        else:
            pz = ctx.enter_context(tc.tile_pool(name="pz", bufs=1, space="PSUM"))
            pxt = ctx.enter_context(tc.tile_pool(
                name="pxt", bufs=1 if l5_3bank else 2, space="PSUM"))
            pxx = ctx.enter_context(tc.tile_pool(
                name="pxx", bufs=2 if l5_3bank else 1, space="PSUM"))
            zxx_tag = "zxx"
        pl5 = ctx.enter_context(tc.tile_pool(name="pl5", bufs=1, space="PSUM"))

        # ---- constants ----
        # L1 needs only wt0 + bias: load those first so pair 0 starts early.
        c_wt0 = consts.tile([128, 64], F16, tag="cwt0")
        nc.sync.dma_start(c_wt0[:], d_wt0[:])
        c_bias = consts.tile([128, 4], F32, tag="cbias")
        nc.sync.dma_start(
            c_bias[:].rearrange("p (l o) -> p l o", o=1),
            d_bias[0:4].transpose([1, 0, 2]))
        # Hoist the first groups' xt DMAs ahead of the big weight-variant
        # DMA: all DMAs share one in-order queue, so pair 0 would otherwise
        # wait ~10us behind the 300KB+ c_wt transfer.
        xt_tiles = {}

        def load_group(g):
            xt_t = sbx.tile([128, NT2], F16, tag="xt")
            nc.sync.dma_start(xt_t[0:8, :], d_xt[g, 0])
            nc.sync.dma_start(xt_t[64:72, :], d_xt[g, 1])
            xt_tiles[g] = xt_t

        for g in range(min(cfg.get("xt_pre", 1), (npair + 1) // 2)):
            load_group(g)
        nwt = 13 if cfg["l1_splitmm"] else 10
        c_wt = consts.tile([128, nwt * 128], F16, tag="cwt")
        # one DMA: src [i, p, c] -> dest [p, i*128 + c]
        nc.sync.dma_start(
            c_wt[:].rearrange("p (i c) -> p i c", i=nwt),
            d_wt[0:nwt].transpose([1, 0, 2]))
        if cfg["l1_splitmm"]:
            c_cxt = consts.tile([1, 256], F16, tag="ccxt")
            nc.sync.dma_start(c_cxt[:], d_cxt[:])
            c_ones = consts.tile([1, NT], F16, tag="cones")
            nc.vector.memset(c_ones[:], 1.0)
        c_w4 = consts.tile([128, 12], F16, tag="cw4")
        nc.sync.dma_start(c_w4[:], d_w4[:])

        def wt_v(i):
            return c_wt[:, i * 128:(i + 1) * 128]

        def w4p(i):
            return c_w4[:, 2 * i:2 * i + 2]

        def tt(eng, out, i0, i1, op):
            if eng == "pool":
                nc.gpsimd.tensor_tensor(out, i0, i1, op)
            else:
                nc.vector.tensor_tensor(out, i0, i1, op)

        def stt(eng, out, i0, s, i1, op0, op1):
            if eng == "pool":
                nc.gpsimd.scalar_tensor_tensor(out, i0, s, i1, op0, op1)
            else:
                nc.vector.scalar_tensor_tensor(out, i0, s, i1, op0, op1)

        def do_pair(xt_t, C, pj, l5):
            """Generator: yields after L1 and after each mid layer so two
            pairs can be interleaved. pj: pair index within L5 block."""
            # ---- L1 ----
            z1 = pz.tile([128, NT], F32, tag="z")
            for sl, tp in ((P0, (0, 0)), (P1, (64, 64))):
                r8 = slice(sl.start, sl.start + 8)
                nc.tensor.matmul(z1[sl, :], c_wt0[r8, 0:64], xt_t[r8, C],
                                 start=True, stop=True, tile_position=tp)
            A = sba.tile([128, NT], F16, tag="A")
            nc.scalar.activation(A[:], z1[:], TANH, bias=c_bias[:, 0:1], scale=1.0)
            A2 = sbb.tile([128, NT], F16, tag="A2")
            if cfg.get("l1_a2_split") and cfg["a2_engine"][0] != "act":
                NW1 = cfg["a2_ways"]
                for wi in range(NW1):
                    sl_ = slice(wi * NT // NW1, (wi + 1) * NT // NW1)
                    tt(cfg["a2_engine"][0], A2[:, sl_], A[:, sl_], A[:, sl_], MUL)
            elif cfg["a2_engine"][0] == "act":
                nc.scalar.activation(A2[:], A[:], SQUARE, bias=0.0, scale=1.0)
            else:
                tt(cfg["a2_engine"][0], A2[:], A[:], A[:], MUL)
            if cfg["l1_splitmm"]:
                A13 = sba.tile([128, NT], F16, tag="A13")
                tt(cfg["g1_engine"], A13[:], A[:], A2[:], MUL)
                S = G1 = None
            elif cfg.get("l1_a2_split"):
                S = sba.tile([128, NT], F16, tag="S")
                NW1 = cfg["a2_ways"]
                for wi in range(NW1):
                    sl_ = slice(wi * NT // NW1, (wi + 1) * NT // NW1)
                    nc.vector.tensor_scalar(S[:, sl_], A2[:, sl_], -1.0, 1.0, MUL, ADD)
                G1 = sba.tile([128, NT], F16, tag="G1")
                tt(cfg["g1_engine"], G1[:], A[:], S[:], MUL)
            else:
                S = sba.tile([128, NT], F16, tag="S")
                nc.vector.tensor_scalar(S[:], A2[:], -1.0, 1.0, MUL, ADD)
                G1 = sba.tile([128, NT], F16, tag="G1")
                tt(cfg["g1_engine"], G1[:], A[:], S[:], MUL)
                A13 = None
            A2_1 = A2
            yield

            # ---- L2..L4 ----
            # xx carries either ("single", rhs, variant) or ("split", W3s, R2s,
            # v_plain, v_neg) describing how to build the NEXT zxx psum.
            xx = ("single", G1, 3)
            XTX = (S, slice(0, NT))
            XTT = (S, slice(0, NT))
            for l in (0, 1, 2):
                if l == 0:
                    vz, vx, vt = 0, 1, 2
                    rz = A
                else:
                    vz, vx, vt = 4 + 2 * (l - 1), 5 + 2 * (l - 1), 5 + 2 * (l - 1)
                    rz = A
                (rx, rxs), (rt, rts) = XTX, XTT
                vp = 4 + 2 * (l - 1)   # plain bd(WT) of this layer (valid l>=1)
                vn = 8 + (l - 1)       # bd(-WT)
                z = pz.tile([128, NT], F32, tag="z")
                zxt = pxt.tile([128, NT2], F32, tag="zxt")
                zxx = pxx.tile([128, NT], F32, tag=zxx_tag)
                zxt_first = cfg.get("zxt_first") and not (l == 0 and cfg["l1_splitmm"])
                if zxt_first:
                    nc.tensor.matmul(zxt[:, 0:NT], wt_v(vx), rx[:, rxs], start=True, stop=True, tile_position=(0, 0))
                    nc.tensor.matmul(zxt[:, NT:NT2], wt_v(vt), rt[:, rts], start=True, stop=True, tile_position=(0, 0))
                nc.tensor.matmul(z[:], wt_v(vz), rz[:, 0:NT], start=True, stop=True, tile_position=(0, 0))
                if l == 0 and cfg["l1_splitmm"]:
                    nc.tensor.matmul(zxt[:, 0:NT], c_cxt[0:1, 0:128], c_ones[0:1, :], start=True, stop=False, tile_position=(0, 0))
                    nc.tensor.matmul(zxt[:, 0:NT], wt_v(10), A2_1[:, 0:NT], start=False, stop=True, tile_position=(0, 0))
                    nc.tensor.matmul(zxt[:, NT:NT2], c_cxt[0:1, 128:256], c_ones[0:1, :], start=True, stop=False, tile_position=(0, 0))
                    nc.tensor.matmul(zxt[:, NT:NT2], wt_v(11), A2_1[:, 0:NT], start=False, stop=True, tile_position=(0, 0))
                    nc.tensor.matmul(zxx[:], wt_v(3), A[:, 0:NT], start=True, stop=False, tile_position=(0, 0))
                    nc.tensor.matmul(zxx[:], wt_v(12), A13[:, 0:NT], start=False, stop=True, tile_position=(0, 0))
                else:
                    if not zxt_first:
                        nc.tensor.matmul(zxt[:, 0:NT], wt_v(vx), rx[:, rxs], start=True, stop=True, tile_position=(0, 0))
                        nc.tensor.matmul(zxt[:, NT:NT2], wt_v(vt), rt[:, rts], start=True, stop=True, tile_position=(0, 0))
                    if xx[0] == "single":
                        nc.tensor.matmul(zxx[:], wt_v(xx[2]), xx[1][:, 0:NT], start=True, stop=True, tile_position=(0, 0))
                    else:
                        nc.tensor.matmul(zxx[:], wt_v(xx[3]), xx[1][:, 0:NT], start=True, stop=False, tile_position=(0, 0))
                        nc.tensor.matmul(zxx[:], wt_v(xx[4]), xx[2][:, 0:NT], start=False, stop=True, tile_position=(0, 0))

                A = sba.tile([128, NT], F16, tag="A")
                nc.scalar.activation(A[:], z[:], TANH, bias=c_bias[:, l + 1:l + 2], scale=1.0)
                w3_fuse = cfg["w3_engine"][l] == "fuse" and not inter
                ZXTt = sbx.tile([128, NT2], F16, tag="ZXT")
                if cfg.get("evac_split"):
                    nc.scalar.activation(ZXTt[:, 0:NT], zxt[:, 0:NT], COPY, bias=0.0, scale=SQRT2)
                    nc.scalar.activation(ZXTt[:, NT:NT2], zxt[:, NT:NT2], COPY, bias=0.0, scale=SQRT2)
                else:
                    nc.scalar.activation(ZXTt[:], zxt[:], COPY, bias=0.0, scale=SQRT2)
                ZXT = ZXTt[:]
                if w3_fuse:
                    zxxs = sbb.tile([128, NT], F16, tag="zxxs")
                    nc.scalar.activation(zxxs[:], zxx[:], COPY, bias=0.0, scale=1.0)
                A2 = sbb.tile([128, NT], F16, tag="A2")
                NW = cfg["a2_ways"]
                NH = NT // NW
                a2sp = cfg["a2_split"] and (cfg.get("a2_split_layers") or (1, 1, 1))[l]
                if a2sp:
                    for wi in range(NW):
                        sl_ = slice(wi * NH, (wi + 1) * NH)
                        tt(cfg["a2_engine"][l + 1], A2[:, sl_], A[:, sl_], A[:, sl_], MUL)
                elif cfg["a2_engine"][l + 1] == "act":
                    nc.scalar.activation(A2[:], A[:], SQUARE, bias=0.0, scale=1.0)
                else:
                    tt(cfg["a2_engine"][l + 1], A2[:], A[:], A[:], MUL)
                splitmm = cfg["xx_mode"][l] == "splitmm"
                H = None
                if splitmm and cfg.get("h_early"):
                    H = sbb.tile([128, NT], F16, tag="H")
                    tt(cfg["h_engine"][l], H[:], A[:], ZXT[:, 0:NT], MUL)
                S = sba.tile([128, NT], F16, tag="S")
                if a2sp:
                    for wi in range(NW):
                        sl_ = slice(wi * NH, (wi + 1) * NH)
                        nc.vector.tensor_scalar(S[:, sl_], A2[:, sl_], -1.0, 1.0, MUL, ADD)
                else:
                    nc.vector.tensor_scalar(S[:], A2[:], -1.0, 1.0, MUL, ADD)
                if cfg["xtt_engine"][l] == "dvew":
                    XT = sbx.tile([128, NT2], F16, tag="XT")
                    if cfg["xt_split"] and (cfg.get("xt_split_layers") or (1, 1, 1))[l]:
                        NQ = NT // 2
                        for wi in range(2):
                            ps_ = slice(wi * NQ, (wi + 1) * NQ)
                            s_rep = S[:, ps_].unsqueeze(1).broadcast_to((128, 2, NQ))
                            nc.vector.tensor_tensor(
                                XT[:].rearrange("p (a b) -> p a b", a=2)[:, :, ps_], s_rep,
                                ZXT[:].rearrange("p (a b) -> p a b", a=2)[:, :, ps_], MUL)
                    else:
                        s_rep = S[:].unsqueeze(1).broadcast_to((128, 2, NT))
                        nc.vector.tensor_tensor(
                            XT[:].rearrange("p (a b) -> p a b", a=2), s_rep,
                            ZXT[:].rearrange("p (a b) -> p a b", a=2), MUL)
                    XTX = (XT, slice(0, NT))
                    XTT = (XT, slice(NT, NT2))
                else:
                    XTxs = sbx.tile([128, NT], F16, tag="XTxs")
                    nc.vector.tensor_tensor(XTxs[:], S[:], ZXT[:, 0:NT], MUL)
                    if cfg.get("xtt_late"):
                        XTts = None   # deferred to end of layer block
                    else:
                        XTts = sbx.tile([128, NT], F16, tag="XTts")
                        tt(cfg["xtt_engine"][l], XTts[:], S[:], ZXT[:, NT:NT2], MUL)
                        XTT = (XTts, slice(0, NT))
                    XTX = (XTxs, slice(0, NT))
                nvp = (4 + 2 * l) if l < 2 else None
                nvn = 8 + l if l < 2 else None
                lpiece = 3
                if splitmm:
                    r2e = (cfg.get("r2_early_layers") or (cfg.get("r2_early"),) * 3)[l]
                    if r2e:
                        if not cfg.get("h_early"):
                            H = sbb.tile([128, NT], F16, tag="H")
                            tt(cfg["h_engine"][l], H[:], A[:], ZXT[:, 0:NT], MUL)
                        R2 = sbb.tile([128, NT], F16, tag="R2")
                        tt(cfg["r2_engine"][l], R2[:], XTX[0][:, XTX[1]], H[:], MUL)
                        W3s = sba.tile([128, NT], F16, tag="W3s")
                        if w3_fuse:
                            nc.vector.tensor_tensor(W3s[:], S[:], zxxs[:], MUL)
                        else:
                            stt(cfg["w3_engine"][l], W3s[:], S[:], 1.0, zxx[:], MUL, MUL)
                        xx = ("split", W3s, R2, nvp, nvn, lpiece)
                    elif cfg.get("w3_last2"):
                        if H is None:
                            H = sbb.tile([128, NT], F16, tag="H")
                            tt(cfg["h_engine"][l], H[:], A[:], ZXT[:, 0:NT], MUL)
                        if cfg.get("r2_before_w3"):
                            R2 = sbb.tile([128, NT], F16, tag="R2")
                            tt(cfg["r2_engine"][l], R2[:], XTX[0][:, XTX[1]], H[:], MUL)
                            W3s = sba.tile([128, NT], F16, tag="W3s")
                            stt(cfg["w3_engine"][l], W3s[:], S[:], 1.0, zxx[:], MUL, MUL)
                        else:
                            W3s = sba.tile([128, NT], F16, tag="W3s")
                            stt(cfg["w3_engine"][l], W3s[:], S[:], 1.0, zxx[:], MUL, MUL)
                            R2 = sbb.tile([128, NT], F16, tag="R2")
                            tt(cfg["r2_engine"][l], R2[:], XTX[0][:, XTX[1]], H[:], MUL)
                        xx = ("split", W3s, R2, nvp, nvn, lpiece)
                    else:
                        W3s = sba.tile([128, NT], F16, tag="W3s")
                        if cfg["w3_engine"][l] == "evac":
                            zxxs = sbb.tile([128, NT], F16, tag="zxxs")
                            nc.scalar.activation(zxxs[:], zxx[:], COPY, bias=0.0, scale=1.0)
                            tt("dve", W3s[:], S[:], zxxs[:], MUL)
                        else:
                            stt(cfg["w3_engine"][l], W3s[:], S[:], 1.0, zxx[:], MUL, MUL)
                        if not cfg.get("h_early"):
                            H = sbb.tile([128, NT], F16, tag="H")
                            tt(cfg["h_engine"][l], H[:], A[:], ZXT[:, 0:NT], MUL)
                        R2 = sbb.tile([128, NT], F16, tag="R2")
                        tt(cfg["r2_engine"][l], R2[:], XTX[0][:, XTX[1]], H[:], MUL)
                        xx = ("split", W3s, R2, nvp, nvn, lpiece)
                else:
                    P = sbb.tile([128, NT], F16, tag="P")
                    if cfg["p_engine"] == "act":
                        nc.scalar.activation(P[:], ZXT[:, 0:NT], SQUARE, bias=0.0, scale=1.0)
                    else:
                        tt(cfg["p_engine"], P[:], ZXT[:, 0:NT], ZXT[:, 0:NT], MUL)
                    M = sbb.tile([128, NT], F16, tag="M")
                    tt(cfg["m_engine"], M[:], A[:], P[:], MUL)
                    INNER = sbb.tile([128, NT], F16, tag="INNER")
                    stt(cfg["inner_engine"], INNER[:], M[:], -1.0, zxx[:], MUL, ADD)
                    AXX = sba.tile([128, NT], F16, tag="AXX")
                    tt(cfg["axx_engine"], AXX[:], S[:], INNER[:], MUL)
                    xx = ("single", AXX, nvp)
                if cfg.get("xtt_late") and cfg["xtt_engine"][l] != "dvew":
                    XTts = sbx.tile([128, NT], F16, tag="XTts")
                    tt(cfg["xtt_engine"][l], XTts[:], S[:], ZXT[:, NT:NT2], MUL)
                    XTT = (XTts, slice(0, NT))
                yield

            # ---- L5 ----
            if cfg["l5_mode"] == "3bank4":
                cp = 32 * pj
                O = slice(cp, cp + 2)
                u_t, ux_t, ux_cp = l5["u"], l5["ux"], cp
            else:
                # 2-bank: u rows {32j}, ux rows {64+32j} of the SAME bank
                cp = 32 * pj
                O = slice(cp, cp + 2)
                u_t, ux_t, ux_cp = l5["u"], l5["u"], 64 + 32 * pj
            OX = slice(ux_cp, ux_cp + 2)
            nc.tensor.matmul(u_t[O, :], w4p(0), A[:, 0:NT], start=True, stop=True, tile_position=(0, cp))
            nc.tensor.matmul(ux_t[OX, :], w4p(1), XTX[0][:, XTX[1]], start=True, stop=True, tile_position=(0, ux_cp))
            if xx[0] == "split":
                nc.tensor.matmul(l5["lin"][O, :], w4p(2), XTT[0][:, XTT[1]], start=True, stop=False, tile_position=(0, cp))
                nc.tensor.matmul(l5["lin"][O, :], w4p(xx[5]), xx[1][:, 0:NT], start=False, stop=False, tile_position=(0, cp))
                nc.tensor.matmul(l5["lin"][O, :], w4p(4), xx[2][:, 0:NT], start=False, stop=True, tile_position=(0, cp))
            else:
                nc.tensor.matmul(l5["lin"][O, :], w4p(2), XTT[0][:, XTT[1]], start=True, stop=False, tile_position=(0, cp))
                nc.tensor.matmul(l5["lin"][O, :], w4p(3), xx[1][:, 0:NT], start=False, stop=True, tile_position=(0, cp))

        def flush_us(l5, nblk):
            R = slice(0, 32 * (nblk - 1) + 2)
            US = sbf.tile([98, NT], F16, tag="US")
            nc.scalar.activation(US[R, :], l5["u"][R, :], COPY, bias=0.0, scale=1.0)
            return US

        def flush_tf(l5, nblk, pair0, US):
            R = slice(0, 32 * (nblk - 1) + 2)
            T1 = sbf.tile([98, NT], F16, tag="T1")
            nc.vector.tensor_tensor(T1[R, :], US[R, :], l5["ux"][R, :], MUL)
            F = sbf.tile([128, NT], F16, tag="F")
            nc.vector.tensor_tensor(F[R, :], T1[R, :], l5["lin"][R, :], ADD)
            for j in range(nblk):
                nc.sync.dma_start(d_f[pair0 + j], F[32 * j:32 * j + 2, :])

        def flush_l5(l5, nblk, pair0):
            R = slice(0, 32 * (nblk - 1) + 2)
            if cfg["l5_mode"] == "3bank4":
                US = sbf.tile([98, NT], F16, tag="US")
                nc.scalar.activation(US[R, :], l5["u"][R, :], COPY, bias=0.0, scale=1.0)
                if cfg["flush_evac"]:
                    UXS = sbf.tile([98, NT], F16, tag="UXS")
                    nc.scalar.activation(UXS[R, :], l5["ux"][R, :], COPY, bias=0.0, scale=1.0)
                    LINS = sbf.tile([98, NT], F16, tag="LINS")
                    nc.scalar.activation(LINS[R, :], l5["lin"][R, :], COPY, bias=0.0, scale=1.0)
                    T1 = sbf.tile([98, NT], F16, tag="T1")
                    tt(cfg["flush_engine"][0], T1[R, :], US[R, :], UXS[R, :], MUL)
                    F = sbf.tile([128, NT], F16, tag="F")
                    tt(cfg["flush_engine"][1], F[R, :], T1[R, :], LINS[R, :], ADD)
                    for j in range(nblk):
                        nc.sync.dma_start(d_f[pair0 + j], F[32 * j:32 * j + 2, :])
                    return
                T1 = sbf.tile([98, NT], F16, tag="T1")
                tt(cfg["flush_engine"][0], T1[R, :], US[R, :], l5["ux"][R, :], MUL)
                F = sbf.tile([128, NT], F16, tag="F")
                tt(cfg["flush_engine"][1], F[R, :], T1[R, :], l5["lin"][R, :], ADD)
            elif cfg["l5_mode"] == "2bank2_act":
                US = sbf.tile([34, NT], F16, tag="US")
                nc.scalar.activation(US[R, :], l5["u"][R, :], COPY, bias=0.0, scale=1.0)
                UXS = sbf.tile([34, NT], F16, tag="UXS")
                nc.scalar.activation(UXS[R, :], l5["u"][64:64 + 32 * (nblk - 1) + 2, :],
                                     COPY, bias=0.0, scale=1.0)
                T1 = sbf.tile([34, NT], F16, tag="T1")
                nc.vector.tensor_tensor(T1[R, :], US[R, :], UXS[R, :], MUL)
                F = sbf.tile([34, NT], F16, tag="F")
                nc.vector.tensor_tensor(F[R, :], T1[R, :], l5["lin"][R, :], ADD)
            else:  # 2bank2_psum
                US = sbf.tile([34, NT], F16, tag="US")
                nc.scalar.activation(US[R, :], l5["u"][R, :], COPY, bias=0.0, scale=1.0)
                T1 = sbf.tile([34, NT], F16, tag="T1")
                nc.vector.tensor_tensor(T1[R, :], US[R, :],
                                        l5["u"][64:64 + 32 * (nblk - 1) + 2, :], MUL)
                F = sbf.tile([34, NT], F16, tag="F")
                nc.vector.tensor_tensor(F[R, :], T1[R, :], l5["lin"][R, :], ADD)
            # rows {32j, 32j+1} -> d_f[pair0 + j]
            for j in range(nblk):
                nc.sync.dma_start(d_f[pair0 + j], F[32 * j:32 * j + 2, :])

        def drive(gens):
            done = [False] * len(gens)
            while not all(done):
                for i, g in enumerate(gens):
                    if not done[i]:
                        try:
                            next(g)
                        except StopIteration:
                            done[i] = True

        LB = 4 if cfg["l5_mode"] == "3bank4" else 2
        pending = None
        pi = 0
        while pi < npair:
            nblk = min(LB, npair - pi)
            l5u = pl5.tile([128, NT], F32, tag="l5u")
            l5lin = pl5.tile([128, NT], F32, tag="l5lin")
            l5 = dict(u=l5u, lin=l5lin)
            if cfg["l5_mode"] == "3bank4":
                l5ux = pl5.tile([128, NT], F32, tag="l5ux")
                l5["ux"] = l5ux
            gens = []
            for j in range(nblk):
                p = pi + j
                g, half = p // 2, p % 2
                if half == 0 and g not in xt_tiles:
                    load_group(g)
                xt_t = xt_tiles[g]
                # pair p uses cols: pair0 -> 0:NT, pair1 -> NT:NT2 of group tiles
                C = slice(half * NT, half * NT + NT)
                gens.append(do_pair(xt_t, C, j, l5))
            if inter:
                drive(gens)
            elif cfg["defer_tf"]:
                next(gens[0])          # L1 of first pair
                if pending is not None:
                    flush_tf(*pending)
                    pending = None
                for g_ in gens:
                    drive([g_])
            else:
                for g_ in gens:
                    drive([g_])
            if cfg["defer_tf"]:
                US = flush_us(l5, nblk)
                pending = (l5, nblk, pi, US)
            else:
                flush_l5(l5, nblk, pi)
            pi += nblk
        if pending is not None:
            flush_tf(*pending)

    nc.compile()
    return nc


def _host_prep(x, t, W0, b0, W1, b1, W2, b2, W3, b3, W4, b4,
               npair=NPAIR, npt_core=NPT_CORE):
    ngroup = (npair + 1) // 2
    pad_core = ngroup * 4 * NT
    n_total = NCORES * npt_core
    xf = np.asarray(x).reshape(-1).astype(np.float32)[:n_total]
    tf = np.asarray(t).reshape(-1).astype(np.float32)[:n_total]

    W0 = np.asarray(W0, np.float32)
    W1 = np.asarray(W1, np.float32)
    W2 = np.asarray(W2, np.float32)
    W3 = np.asarray(W3, np.float32)
    W4 = np.asarray(W4, np.float32)
    b4v = float(np.asarray(b4).reshape(-1)[0])
    s2 = np.float32(np.sqrt(2.0))

    xcol = W0[:, 0]
    tccol = W0[:, 1] + np.float32(b4v) * W0[:, 0]   # t~ seed folds b4*u_x
    ccol = -2.0 * W0[:, 0] ** 2

    def bd(M):
        out = np.zeros((128, 128), np.float16)
        out[0:64, 0:64] = M.astype(np.float16)
        out[64:128, 64:128] = M.astype(np.float16)
        return out

    wt = np.zeros((13, 128, 128), np.float16)
    wt[0] = bd(W1.T)
    wt[1] = bd(xcol[:, None] * W1.T)
    wt[2] = bd(tccol[:, None] * W1.T)
    wt[3] = bd(ccol[:, None] * W1.T)
    wt[4] = bd(W2.T)
    wt[5] = bd(W2.T / s2)
    wt[6] = bd(W3.T)
    wt[7] = bd(W3.T / s2)
    wt[8] = bd(-W2.T)
    wt[9] = bd(-W3.T)
    wt[10] = bd(-(xcol[:, None] * W1.T))
    wt[11] = bd(-(tccol[:, None] * W1.T))
    wt[12] = bd(-(ccol[:, None] * W1.T))

    # K=1 const rows: cx = W1 @ (diag(xcol) 1) = W1 @ xcol, ct = W1 @ tccol
    cx = (W1 @ xcol).astype(np.float16)
    ct = (W1 @ tccol).astype(np.float16)
    cxt = np.zeros((1, 256), np.float16)
    cxt[0, 0:64] = cx
    cxt[0, 64:128] = cx
    cxt[0, 128:192] = ct
    cxt[0, 192:256] = ct

    # L1 exact product lhsT (same as v1)
    W0Thi, W0Tlo = _split16(W0.T)
    wt0_half = np.concatenate([W0Thi, W0Tlo, W0Tlo, W0Thi], 0)  # [8, 64]
    wt0 = np.zeros((128, 64), np.float16)
    wt0[0:8] = wt0_half
    wt0[64:72] = wt0_half

    # L5 pieces: bd2(v) = [128,2]: col0 = [v;0], col1 = [0;v]
    def bd2(v):
        out = np.zeros((128, 2), np.float16)
        out[0:64, 0] = v.astype(np.float16)
        out[64:128, 1] = v.astype(np.float16)
        return out

    w4r = W4.reshape(-1)
    w4 = np.concatenate([
        bd2(w4r),                      # u
        bd2(w4r / s2),                 # ux
        bd2(w4r / s2),                 # lin: at~ piece
        bd2(np.float32(-NU) * w4r),    # lin: xx piece (W3s plain or AXX)
        bd2(np.float32(NU) * w4r),     # lin: xx R2 piece (splitmm)
        bd2(np.float32(-NU) * w4r / s2),  # lin: xx piece for sqrt2-scaled W3s
    ], axis=1)  # [128, 12]

    def dup_col(v):
        out = np.zeros((128, 1), np.float32)
        out[0:64, 0] = v
        out[64:128, 0] = v
        return out

    bias = np.stack([dup_col(np.asarray(b, np.float32).reshape(-1))
                     for b in (b0, b1, b2, b3)])

    in_maps = []
    for c in range(NCORES):
        xs = np.zeros(pad_core, np.float32)
        ts_ = np.zeros(pad_core, np.float32)
        xs[:npt_core] = xf[c * npt_core:(c + 1) * npt_core]
        ts_[:npt_core] = tf[c * npt_core:(c + 1) * npt_core]
        xhi, xlo = _split16(xs)
        thi, tlo = _split16(ts_)
        rows = np.stack([xhi, thi, xlo, tlo, xhi, thi, xlo, tlo])  # [8, pad]
        r4 = rows.reshape(8, ngroup, 4, NT)  # tiles: A0,B0,A1,B1
        xt = np.zeros((ngroup, 2, 8, 2 * NT), np.float16)
        xt[:, 0, :, 0:NT] = np.transpose(r4[:, :, 0], (1, 0, 2))
        xt[:, 0, :, NT:] = np.transpose(r4[:, :, 2], (1, 0, 2))
        xt[:, 1, :, 0:NT] = np.transpose(r4[:, :, 1], (1, 0, 2))
        xt[:, 1, :, NT:] = np.transpose(r4[:, :, 3], (1, 0, 2))
        in_maps.append(dict(xt=xt, wt=wt, wt0=wt0, w4=w4, bias=bias, cxt=cxt))
    return in_maps


def _gather(results, npair=NPAIR, npt_core=NPT_CORE):
    outs = []
    for c in range(NCORES):
        f = results[c]["f"].astype(np.float32)  # [npair, 2, NT]
        # pair p covers: row0 = tile A (pts 4g*NT + ...), row1 = tile B.
        # point order per group g: A0, B0, A1, B1 each NT:
        #   pair (g,0): rows (A0, B0); pair (g,1): rows (A1, B1)
        f = f.reshape(npair // 2, 2, 2, NT)       # [g, half, AB, NT]
        f = np.transpose(f, (0, 1, 2, 3)).reshape(npair // 2, 4 * NT)
        outs.append(f.reshape(-1)[:npt_core])
    return np.concatenate(outs)[:, None]


_CACHED_NC = None


def kernel(**inputs):
    global _CACHED_NC
    import sys
    if "/opt/trn_rl_repo" not in sys.path:
        sys.path.insert(0, "/opt/trn_rl_repo")
    from concourse.bass_utils import run_bass_kernel_spmd

    if _CACHED_NC is None:
        _CACHED_NC = _build_program()
    nc = _CACHED_NC
    in_maps = _host_prep(**inputs)
    res = run_bass_kernel_spmd(nc, in_maps, list(range(NCORES)))
    return _gather(res.results)


if __name__ == "__main__":
    rng = np.random.default_rng(0)
    LAYERS = [2, 64, 64, 64, 64, 1]
    inp = dict(
        x=rng.standard_normal((500000, 1)).astype(np.float32),
        t=rng.random((500000, 1)).astype(np.float32),
    )
    for i in range(5):
        inp[f"W{i}"] = (rng.standard_normal((LAYERS[i + 1], LAYERS[i]))
                        / np.sqrt(LAYERS[i])).astype(np.float32)
        inp[f"b{i}"] = np.zeros(LAYERS[i + 1], np.float32)
    out = kernel(**inp)
    print("out", out.shape, out.dtype, np.abs(out).max())

